# revision 1
# baseline (speedup 1.0000x reference)
"""Multi-head attention (BaselineAttention) Bass kernel for 8 trn2 NeuronCores.

Problem: x[4,2048,1024], per-head Wq/Wk/Wv [16,1024,64] (+biases), Wo[1024,1024]+bo.
Sharding: core c -> batch b=c//2, head-group g=c%2 (8 heads each).
Each core computes y_partial[b] = sum_{h in group} softmax(qk^T/8) v @ Wo_rows(h).
Host combines: y[b] = part[2b] + part[2b+1] + bo + bv@Wo  (bv folded out of device).

Device algorithm per core (all matmul operands bf16; psum f32):
  x resident in SBUF [128, 8kt, 2048]; per pair (2 heads): qT/kT [128=(j,e), s]
  via W^T@x; v[t,(j,e)] via x^T@Wv with an appended ones column (vA [.,tt,j,65]).
  Attention per (head, s-chunk 512): scores^T[t,s] = kT^T qT (K=64); exp on ACT
  -> att bf16 [t, s]; o_aug[s, e|r] = att^T-stationary @ [v|1]-moving (N=65,
  psum-accumulated over 16 t-tiles); normalize with DVE per-partition scalar
  1/r; PE-transpose o_n -> onorm[(j,e), pair, s]; out-proj y = onorm^T @ Wo.
  Pair-pipelined schedule: QKV of pair p+1 and out-proj fill PE while ACT
  computes exp of pair p (exp is the co-bottleneck at ~267us vs PE ~285us).
"""
import numpy as np

B, S, DIM, H, DH = 4, 2048, 1024, 16, 64
NCORES = 8
HPC = H // 2          # heads per core = 8
NPAIR = HPC // 2      # head pairs per core = 4
NT = S // 128         # t-tiles = 16
NSQ = S // 512        # s-chunks of 512 = 4
NKT = DIM // 128      # d-tiles = 8
SCALE = 1.0 / float(np.sqrt(DH))

_CACHE = {}


def _build(repeat=1, debug_taps=False, exp_ns=1040, attnv_ns=1830, defer_q=True, greedy_m=-600):
    from collections import deque
    import concourse.bass as bass  # noqa: F401
    import concourse.mybir as mybir
    import concourse.tile as tile
    from concourse import bacc

    f32 = mybir.dt.float32
    bf16 = mybir.dt.bfloat16
    AF = mybir.ActivationFunctionType

    nc = bacc.Bacc("TRN2", target_bir_lowering=False, debug=False,
                   num_devices=NCORES)

    xT_d = nc.dram_tensor("xT", [DIM, S], bf16, kind="ExternalInput")
    wq_d = nc.dram_tensor("wq", [NPAIR, 128, NKT, 128], bf16, kind="ExternalInput")
    wk_d = nc.dram_tensor("wk", [NPAIR, 128, NKT, 128], bf16, kind="ExternalInput")
    wv_d = nc.dram_tensor("wv", [NPAIR, 128, NKT, 128], bf16, kind="ExternalInput")
    bq_d = nc.dram_tensor("bq", [128, NPAIR], f32, kind="ExternalInput")
    bk_d = nc.dram_tensor("bk", [128, NPAIR], f32, kind="ExternalInput")
    wo_d = nc.dram_tensor("wo", [128, NPAIR, DIM], bf16, kind="ExternalInput")
    eye_d = nc.dram_tensor("eye", [128, 128], bf16, kind="ExternalInput")
    y_d = nc.dram_tensor("y", [S, DIM], f32, kind="ExternalOutput")
    if debug_taps:
        dbg_qT = nc.dram_tensor("dbg_qT", [128, S], bf16, kind="ExternalOutput")
        dbg_kT = nc.dram_tensor("dbg_kT", [128, S], bf16, kind="ExternalOutput")
        dbg_vA = nc.dram_tensor("dbg_vA", [128, NT, 2, 65], bf16,
                                kind="ExternalOutput")
        dbg_att = nc.dram_tensor("dbg_att", [128, NT, 512], bf16,
                                 kind="ExternalOutput")
        dbg_on = nc.dram_tensor("dbg_on", [128, NPAIR, S], bf16,
                                kind="ExternalOutput")

    ctr = [0]

    def nm(pfx):
        ctr[0] += 1
        return f"{pfx}_{ctr[0]}"

    with tile.TileContext(nc) as tc:
        with tc.tile_pool(name="persist", bufs=1) as pp, \
             tc.tile_pool(name="work", bufs=1) as wp, \
             tc.tile_pool(name="ps", bufs=1, space="PSUM") as ps:
            # ---- persistent SBUF ----
            x_sb = pp.tile([128, NKT, S], bf16)
            eye = pp.tile([128, 128], bf16)
            bqs = pp.tile([128, NPAIR], f32)
            bks = pp.tile([128, NPAIR], f32)
            wo_sb = pp.tile([128, NPAIR, DIM], bf16)
            onorm = pp.tile([128, NPAIR, S], bf16)   # [(j,e), pair, s]

            for rep in range(repeat):
                xT_src = xT_d.ap().rearrange("(kt p) s -> p kt s", p=128)
                nc.sync.dma_start(out=x_sb[:, 0, :], in_=xT_src[:, 0, :])

                wtiles = {}

                def w_dmas(p, only=None):
                    for pfx, wd in (("k", wk_d), ("q", wq_d), ("v", wv_d)):
                        if only and pfx not in only:
                            continue
                        w = wp.tile([128, NKT, 128], bf16, tag=f"w{pfx}",
                                    bufs=2, name=nm(f"w{pfx}{p}"))
                        nc.sync.dma_start(out=w, in_=wd.ap()[p])
                        wtiles[(pfx, p)] = w

                qkts = {}

                def pair_tiles(p):
                    qT = wp.tile([128, S], bf16, tag="qT", bufs=2,
                                 name=nm(f"qT{p}"))
                    kT = wp.tile([128, S], bf16, tag="kT", bufs=2,
                                 name=nm(f"kT{p}"))
                    vA = wp.tile([128, NT, 2, 65], bf16, tag="vA", bufs=2,
                                 name=nm(f"vA{p}"))
                    qkts[p] = (qT, kT, vA)

                def ones_atom(p):
                    def go():
                        nc.gpsimd.memset(qkts[p][2][:, :, :, 64:65], 1.0)
                    return go

                def qk_atoms(pfx, p, sq):
                    """Two ~850ns PE atoms (4 kt-matmuls each) + bias copy."""
                    cell = {}

                    def a1():
                        dst, bias = ((qkts[p][0], bqs) if pfx == "q"
                                     else (qkts[p][1], bks))
                        cell["ps"] = ps.tile([128, 512], f32, tag="qk",
                                             bufs=1, name=nm(f"ps{pfx}{p}{sq}"))
                        cell["dst"], cell["bias"] = dst, bias
                        w = wtiles[(pfx, p)]
                        for kt in range(4):
                            nc.tensor.matmul(
                                cell["ps"], w[:, kt, :],
                                x_sb[:, kt, sq * 512:(sq + 1) * 512],
                                start=(kt == 0), stop=False)

                    def a2():
                        w = wtiles[(pfx, p)]
                        for kt in range(4, NKT):
                            nc.tensor.matmul(
                                cell["ps"], w[:, kt, :],
                                x_sb[:, kt, sq * 512:(sq + 1) * 512],
                                start=False, stop=(kt == NKT - 1))
                        nc.vector.tensor_scalar_add(
                            out=cell["dst"][:, sq * 512:(sq + 1) * 512],
                            in0=cell["ps"], scalar1=cell["bias"][:, p:p + 1])
                    return [a1, a2]

                def v_atoms(p, ttg):
                    """Two ~850ns PE atoms (2 t-tiles each) + copy to vA."""
                    cell = {}

                    def half(h):
                        def go():
                            if h == 0:
                                cell["ps"] = ps.tile(
                                    [128, 512], f32, tag="vps", bufs=1,
                                    name=nm(f"psv{p}{ttg}"))
                            v_ps4 = cell["ps"].rearrange("p (t m) -> p t m",
                                                         t=4)
                            w = wtiles[("v", p)]
                            for t4 in (0, 1) if h == 0 else (2, 3):
                                tt = ttg * 4 + t4
                                for kt in range(NKT):
                                    nc.tensor.matmul(
                                        v_ps4[:, t4, :],
                                        x_sb[:, kt, tt * 128:(tt + 1) * 128],
                                        w[:, kt, :],
                                        start=(kt == 0), stop=(kt == NKT - 1))
                            if h == 1:
                                vA = qkts[p][2]
                                nc.vector.tensor_copy(
                                    vA[:, ttg * 4:(ttg + 1) * 4, :, 0:64],
                                    cell["ps"].rearrange(
                                        "p (t j e) -> p t j e", t=4, j=2))
                        return go
                    return [half(0), half(1)]

                def qkv_atoms(p):
                    """(resource_key, cost_ns, closure) atoms for pair p."""
                    pair_tiles(p)
                    out = [((p, "v"), 100, ones_atom(p))]
                    for ttg in range(4):
                        out += [((p, "v"), 880, a) for a in v_atoms(p, ttg)]
                    for sq in range(NSQ):
                        out += [((p, "k"), 880, a)
                                for a in qk_atoms("k", p, sq)]
                    for sq in range(NSQ):
                        out += [((p, f"q{sq}"), 880, a)
                                for a in qk_atoms("q", p, sq)]
                    return out

                # ---- greedy emission scheduler ----
                # Estimated engine clocks (ns) maintained during emission;
                # cold filler (QKV / out-proj atoms) is emitted only while
                # it cannot starve ACT of score tiles.
                GREEDY_M = greedy_m
                SC_MM = 217       # one [128,512] scores matmul
                EXP = exp_ns      # one [128,1024] exp + init
                ATTNV = attnv_ns  # 64 x 65-col matmuls
                TRANSP = 320
                est = {"pe": 0.0, "act": 0.0}
                exp_hist = [0.0, 0.0]   # completion time per sc-tile rotation

                hot = deque()
                cold = deque()

                def pop_cold_one():
                    key, cost, fn = cold.popleft()
                    fn()
                    est["pe"] += cost

                def force_res(*keys):
                    """Emit every cold atom up to and incl. the given
                    resource keys (FIFO order preserved)."""
                    while any(k for (k, c, f) in cold if k in keys):
                        pop_cold_one()

                def greedy_cold():
                    while cold and (est["pe"] + cold[0][1] + 2 * SC_MM
                                    <= est["act"] + GREEDY_M):
                        pop_cold_one()

                def oproj_st(st):
                    out = []
                    if True:
                        for nh in range(2):
                            def go(st=st, nh=nh):
                                tag = "qk" if nh == 0 else "vps"
                                y_ps = ps.tile([128, 512], f32, tag=tag,
                                               bufs=1, name=nm(f"psy{st}{nh}"))
                                for p in range(NPAIR):
                                    nc.tensor.matmul(
                                        y_ps,
                                        onorm[:, p, st * 128:(st + 1) * 128],
                                        wo_sb[:, p, nh * 512:(nh + 1) * 512],
                                        start=(p == 0), stop=(p == NPAIR - 1))
                                y_sb = wp.tile([128, 512], f32, tag="ysb",
                                               bufs=4, name=nm(f"ysb{st}{nh}"))
                                nc.vector.tensor_copy(y_sb, y_ps)
                                nc.sync.dma_start(
                                    out=y_d.ap()[st * 128:(st + 1) * 128,
                                                 nh * 512:(nh + 1) * 512],
                                    in_=y_sb)
                            out.append(("oproj", 880, go))
                    return out

                def attn_back(p, j, sq, att_t, last_slot):
                    """attn@v + normalize + transpose (+ copy & oproj when
                    this closes a (pair, sq) group)."""
                    def go():
                        qT, kT, vA = qkts[p]
                        o_ps = ps.tile([128, 4, 65], f32, tag="ops", bufs=1,
                                       name=nm(f"o{p}{j}{sq}"))
                        for sti in range(4):
                            for tt in range(NT):
                                nc.tensor.matmul(
                                    o_ps[:, sti, :],
                                    att_t[:, tt, sti * 128:(sti + 1) * 128],
                                    vA[:, tt, j, :],
                                    start=(tt == 0), stop=(tt == NT - 1))
                        rinv = wp.tile([128, 4], f32, tag="rinv", bufs=2,
                                       name=nm(f"ri{p}{j}{sq}"))
                        nc.vector.reciprocal(rinv, o_ps[:, :, 64])
                        o_n = wp.tile([128, 4, 64], bf16, tag="on", bufs=2,
                                      name=nm(f"on{p}{j}{sq}"))
                        for sti in range(4):
                            nc.vector.tensor_scalar_mul(
                                out=o_n[:, sti, :],
                                in0=o_ps[:, sti, 0:64],
                                scalar1=rinv[:, sti:sti + 1])
                        pt = pt_for[(p, sq)]
                        for sti in range(4):
                            nc.tensor.matmul(
                                pt[64 * j:64 * (j + 1), sti, :],
                                o_n[:, sti, :], eye, is_transpose=True)
                        if j == 1:
                            if last_slot:
                                for sti in range(4):
                                    st = sq * 4 + sti
                                    nc.vector.tensor_copy(
                                        onorm[:, p, st * 128:(st + 1) * 128],
                                        pt[:, sti, :])
                                    cold.extend(oproj_st(st))
                            else:
                                nc.vector.tensor_copy(
                                    onorm[:, p, sq * 512:(sq + 1) * 512],
                                    pt.rearrange("p st s -> p (st s)"))
                    return go

                def startup_pair0():
                    """kt-outer k-proj + q(sq0) + v(ttg0) emitted per
                    arriving x chunk; v ttg1-3 serial after; q sq1-3 to
                    cold. Attention(p0) can then start ~16us in."""
                    pair_tiles(0)
                    qT, kT, vA = qkts[0]
                    nc.gpsimd.memset(vA[:, :, :, 64:65], 1.0)
                    kps = [ps.tile([128, 2, 512], f32, tag="sc", bufs=2,
                                   name=nm(f"kp0{i}")) for i in range(2)]
                    q_ps = ps.tile([128, 512], f32, tag="qk", bufs=1,
                                   name=nm("psq00"))
                    wk, wq, wv = (wtiles[("k", 0)], wtiles[("q", 0)],
                                  wtiles[("v", 0)])
                    for r in range(NKT + 3):
                        if r < NKT:
                            st_, sp_ = r == 0, r == NKT - 1
                            for sqh in range(NSQ):
                                nc.tensor.matmul(
                                    kps[sqh // 2][:, sqh % 2, :], wk[:, r, :],
                                    x_sb[:, r, sqh * 512:(sqh + 1) * 512],
                                    start=st_, stop=sp_)
                        if r >= 3:
                            kt = r - 3
                            st_, sp_ = kt == 0, kt == NKT - 1
                            nc.tensor.matmul(q_ps, wq[:, kt, :],
                                             x_sb[:, kt, 0:512],
                                             start=st_, stop=sp_)

                    # startup bias-adds: k chunk for the first scores, then q,
                    # then the rest (DVE; keeps ACT's exp table resident)
                    nc.vector.tensor_scalar_add(
                        out=kT[:, 0:512], in0=kps[0][:, 0, :],
                        scalar1=bks[:, 0:1])
                    nc.vector.tensor_scalar_add(
                        out=qT[:, 0:512], in0=q_ps, scalar1=bqs[:, 0:1])
                    nc.vector.tensor_scalar_add(
                        out=kT[:, 512:1024], in0=kps[0][:, 1, :],
                        scalar1=bks[:, 0:1])
                    nc.vector.tensor_scalar_add(
                        out=kT[:, 1024:2048],
                        in0=kps[1].rearrange("p a b -> p (a b)"),
                        scalar1=bks[:, 0:1])
                    for ttg in (0, 1, 2, 3):
                        cold.extend(((0, "v"), 880, a)
                                    for a in v_atoms(0, ttg))
                    for sq in range(1, NSQ):
                        cold.extend(((0, f"q{sq}"), 880, a)
                                    for a in qk_atoms("q", 0, sq))
                    est["pe"] = 16000.0
                    est["act"] = 16000.0

                pt_for = {}

                w_dmas(0, only=("k",))
                if rep == 0:
                    nc.sync.dma_start(out=bqs, in_=bq_d.ap())
                    nc.sync.dma_start(out=bks, in_=bk_d.ap())
                    nc.sync.dma_start(out=eye, in_=eye_d.ap())
                for kt in range(1, 3):
                    nc.sync.dma_start(out=x_sb[:, kt, :], in_=xT_src[:, kt, :])
                w_dmas(0, only=("q", "v"))
                for kt in range(3, NKT):
                    nc.sync.dma_start(out=x_sb[:, kt, :], in_=xT_src[:, kt, :])
                if rep == 0:
                    nc.sync.dma_start(out=wo_sb, in_=wo_d.ap())
                startup_pair0()
                for slot in range(1, NPAIR + 1):
                    if slot < NPAIR:
                        w_dmas(slot)
                        atoms = qkv_atoms(slot)
                        if defer_q and slot == NPAIR - 1:
                            keep = [a for a in atoms
                                    if a[0][1] in ("v", "k", "q0")]
                            deferred = [a for a in atoms if a not in keep]
                            cold.extend(keep)
                        else:
                            cold.extend(atoms)
                    if defer_q and slot == NPAIR:
                        cold.extend(deferred)
                    p = slot - 1
                    last_slot = slot == NPAIR
                    for sq in range(NSQ):
                        pt_for[(p, sq)] = ps.tile(
                            [128, 4, 128], bf16, tag="pt", bufs=1,
                            name=nm(f"pt{p}{sq}"))
                        for j in range(2):
                            force_res((p, "k"), (p, f"q{sq}"))
                            qT, kT, vA = qkts[p]
                            att_t = wp.tile([128, NT, 512], bf16, tag="att",
                                            bufs=3, name=nm(f"att{p}{j}{sq}"))
                            lo = 64 * j
                            for g in range(NT // 2):
                                sc = ps.tile([128, 2, 512], f32, tag="sc",
                                             bufs=2,
                                             name=nm(f"sc{p}{j}{sq}{g}"))
                                dep = exp_hist[-2]
                                for i in range(2):
                                    tt = 2 * g + i
                                    nc.tensor.matmul(
                                        sc[:, i, :],
                                        kT[lo:lo + 64,
                                           tt * 128:(tt + 1) * 128],
                                        qT[lo:lo + 64,
                                           sq * 512:(sq + 1) * 512],
                                        start=True, stop=True)
                                est["pe"] = max(est["pe"] + 2 * SC_MM,
                                                dep + 150 + 2 * SC_MM)
                                nc.scalar.activation(
                                    att_t[:, 2 * g:2 * g + 2, :], sc,
                                    AF.Exp, scale=SCALE)
                                est["act"] = max(est["act"],
                                                 est["pe"] + 150) + EXP
                                exp_hist.append(est["act"])
                                if g == 3 and hot:
                                    hp, hfn = hot.popleft()
                                    force_res((hp, "v"))
                                    hfn()
                                    if len(exp_hist) >= 5:
                                        est["pe"] = max(est["pe"],
                                                        exp_hist[-5] + 150)
                                    est["pe"] += ATTNV + TRANSP
                                else:
                                    greedy_cold()
                            hot.append((p, attn_back(p, j, sq, att_t,
                                                     last_slot)))
                            if debug_taps and p == 0 and j == 0 and sq == 0:
                                def dbg0(att_t=att_t):
                                    force_res((0, "k"), (0, "v"), (0, "q1"),
                                              (0, "q2"), (0, "q3"))
                                    qT0, kT0, vA0 = qkts[0]
                                    nc.sync.dma_start(out=dbg_qT.ap(), in_=qT0)
                                    nc.sync.dma_start(out=dbg_kT.ap(), in_=kT0)
                                    nc.sync.dma_start(out=dbg_vA.ap(), in_=vA0)
                                    nc.sync.dma_start(out=dbg_att.ap(),
                                                      in_=att_t)
                                hot.append((0, dbg0))
                while hot:
                    hp, hfn = hot.popleft()
                    force_res((hp, "v"))
                    hfn()
                while cold:
                    pop_cold_one()
                if debug_taps:
                    nc.sync.dma_start(out=dbg_on.ap(), in_=onorm)
    nc.compile()
    return nc


def _get_nc():
    if "nc" not in _CACHE:
        _CACHE["nc"] = _build()
    return _CACHE["nc"]


def _bf16(a):
    import ml_dtypes
    return np.ascontiguousarray(a).astype(ml_dtypes.bfloat16)


def make_in_maps(x, Wq, Wk, Wv, bq, bk, bv, Wo, bo):
    eye = np.eye(128, dtype=np.float32)
    in_maps = []
    for c in range(NCORES):
        b, g = c // 2, c % 2
        hs = slice(g * HPC, (g + 1) * HPC)
        # weights pair-packed + sbuf-contiguous: [pair, 128pp, NKT, 128=(j,e)]
        def wprep(W):
            w3 = W[hs].reshape(NPAIR, 2, DIM, DH).transpose(0, 2, 1, 3) \
                .reshape(NPAIR, NKT, 128, 128).transpose(0, 2, 1, 3)
            return np.ascontiguousarray(w3)
        wq3, wk3, wv3 = wprep(Wq), wprep(Wk), wprep(Wv)
        # wo: [128=(j,e), pair, DIM]
        wo3 = Wo[g * 512:(g + 1) * 512, :].reshape(NPAIR, 128, DIM) \
            .transpose(1, 0, 2)
        in_maps.append({
            "xT": _bf16(x[b].T),
            "wq": _bf16(wq3),
            "wk": _bf16(wk3),
            "wv": _bf16(wv3),
            "bq": np.ascontiguousarray(bq[hs].reshape(NPAIR, 128).T),
            "bk": np.ascontiguousarray(bk[hs].reshape(NPAIR, 128).T),
            "wo": _bf16(wo3),
            "eye": _bf16(eye),
        })
    return in_maps


def combine(results, bv, Wo, bo):
    const = bv.reshape(DIM) @ Wo + bo          # [DIM]
    y = np.empty((B, S, DIM), dtype=np.float32)
    for b in range(B):
        y[b] = (results[2 * b]["y"].astype(np.float32)
                + results[2 * b + 1]["y"].astype(np.float32) + const)
    return y


def kernel(x, Wq, Wk, Wv, bq, bk, bv, Wo, bo):
    import time
    from concourse.bass_utils import run_bass_kernel_spmd
    x, Wq, Wk, Wv, bq, bk, bv, Wo, bo = [
        np.asarray(a, dtype=np.float32)
        for a in (x, Wq, Wk, Wv, bq, bk, bv, Wo, bo)]
    nc = _get_nc()
    in_maps = make_in_maps(x, Wq, Wk, Wv, bq, bk, bv, Wo, bo)
    last = None
    for attempt in range(3):
        try:
            res = run_bass_kernel_spmd(nc, in_maps,
                                       core_ids=list(range(NCORES)))
            return combine(res.results, bv, Wo, bo)
        except Exception as e:  # transient NRT_EXEC_UNIT_UNRECOVERABLE wedges
            last = e
            time.sleep(75)
    raise last



# revision 20
# speedup vs baseline: 1.1278x; 1.1278x over previous
"""Multi-head attention (BaselineAttention) Bass kernel for 8 trn2 NeuronCores.

Problem: x[4,2048,1024], per-head Wq/Wk/Wv [16,1024,64] (+biases), Wo[1024,1024]+bo.
Sharding: core c -> batch b=c//2, head-group g=c%2 (8 heads each).
Each core computes y_partial[b] = sum_{h in group} softmax(qk^T/8) v @ Wo_rows(h).
Host combines: y[b] = part[2b] + part[2b+1] + bo + bv@Wo  (bv folded out of device).

v2 design (fp8e4 + DoubleRow; cost model charges DR matmuls 0.5 cyc/out-elem):
  - x and Wq/Wk/Wv are host-split into fp8 hi+lo pairs; QKV projections run as
    3-chain DR matmuls (hi*hi + hi*lo + lo*hi) - bf16-accurate at 75% of the
    bf16 PE cost.
  - q/k: psum -> (+bias, ->fp8) qT/kT [128=(j,e), s] -> sbuf->sbuf DMA
    reshuffle to qdr/kdr [32-part e_hi, 2 e_lo, s]; scores are DR matmuls
    contracting (e_hi partition, e_lo pair): half the bf16 scores cost.
  - exp split across ACT (real Exp -> fp8 att) and DVE (one-op Schraudolph:
    round(A*s + B) -> uint8 bit pattern == fp8 att), est-clock balanced.
  - attn@v: DR over t-tile pairs; v stored hi+lo fp8 (v-split kills the
    v-quantization noise); ones column in hi gives r for normalization.
  - normalize via DVE reciprocal + stride-0-broadcast multiply; PE transpose;
    out-proj stays bf16 for accuracy.
"""
import numpy as np

B, S, DIM, H, DH = 4, 2048, 1024, 16, 64
NCORES = 8
HPC = H // 2          # heads per core = 8
NPAIR = HPC // 2      # head pairs per core = 4
NT = S // 128         # t-tiles = 16
NSQ = S // 512        # s-chunks of 512 = 4
NKT = DIM // 128      # d-tiles = 8
NKP = NKT // 2        # d-tile pairs = 4
SCALE = 1.0 / float(np.sqrt(DH))
VW = 80               # padded vA row width (pair step 160B % 16 == 0)

# attnv mode: "split8" (v hi+lo fp8, most accurate fp8 path),
# "plain8" (v single fp8), "bf16" (attn@v in bf16, att bf16)
ATTNV = "split8"

_CACHE = {}


def _build(attnv=ATTNV, exp_margin=0, greedy_m=-600, debug_taps=False):
    from collections import deque
    import concourse.bass as bass  # noqa: F401
    import concourse.mybir as mybir
    import concourse.tile as tile
    from concourse import bacc

    f32 = mybir.dt.float32
    bf16 = mybir.dt.bfloat16
    fp8 = mybir.dt.float8e4
    u8 = mybir.dt.uint8
    u16 = mybir.dt.uint16
    AF = mybir.ActivationFunctionType
    DRM = mybir.MatmulPerfMode.DoubleRow
    split = attnv == "split8"
    att8 = attnv in ("split8", "plain8")
    att_dt = fp8 if att8 else bf16
    NHL = 2 if split else 1

    # Schraudolph exp constants (DVE f32->uint conversion is round-to-nearest)
    A8 = float(8.0 * SCALE / np.log(2.0))
    B8 = float(7 * 8 - 0.344)
    A16 = float(128.0 * SCALE / np.log(2.0))
    B16 = float(127 * 128 - 5.51)

    nc = bacc.Bacc("TRN2", target_bir_lowering=False, debug=False,
                   num_devices=NCORES)

    xh_d = nc.dram_tensor("xh", [DIM, S], fp8, kind="ExternalInput")
    xl_d = nc.dram_tensor("xl", [DIM, S], fp8, kind="ExternalInput")
    xm_d = nc.dram_tensor("xm", [DIM, S], fp8, kind="ExternalInput")
    w_d = {}
    for pfx in ("q", "k", "v"):
        for hl in ("h", "l"):
            w_d[pfx + hl] = nc.dram_tensor(
                f"w{pfx}{hl}", [NPAIR, 128, NKT, 128], fp8,
                kind="ExternalInput")
    bq_d = nc.dram_tensor("bq", [128, NPAIR], f32, kind="ExternalInput")
    bk_d = nc.dram_tensor("bk", [128, NPAIR], f32, kind="ExternalInput")
    wo_d = nc.dram_tensor("wo", [128, NPAIR, DIM], bf16, kind="ExternalInput")
    eye_d = nc.dram_tensor("eye", [128, 128], bf16, kind="ExternalInput")
    y_d = nc.dram_tensor("y", [S, DIM], f32, kind="ExternalOutput")
    if debug_taps:
        dbg = {
            "qT": nc.dram_tensor("dbg_qT", [128, S], fp8,
                                 kind="ExternalOutput"),
            "kT": nc.dram_tensor("dbg_kT", [128, S], fp8,
                                 kind="ExternalOutput"),
            "qdr": nc.dram_tensor("dbg_qdr", [64, 2, S], fp8,
                                  kind="ExternalOutput"),
            "kdr": nc.dram_tensor("dbg_kdr", [64, 2, S], fp8,
                                  kind="ExternalOutput"),
            "att0": nc.dram_tensor("dbg_att0", [128, NT, 512], att_dt,
                                   kind="ExternalOutput"),
            "att1": nc.dram_tensor("dbg_att1", [128, NT, 512], att_dt,
                                   kind="ExternalOutput"),
            "vA": nc.dram_tensor("dbg_vA",
                                 [128, NT, NHL, 2, VW if att8 else 72],
                                 fp8 if att8 else bf16,
                                 kind="ExternalOutput"),
            "vA1": nc.dram_tensor("dbg_vA1", [128, NT, NHL, 2],
                                  fp8 if att8 else bf16,
                                  kind="ExternalOutput"),
            "on": nc.dram_tensor("dbg_on", [128, NPAIR, S], bf16,
                                 kind="ExternalOutput"),
        }
    exp_engines = []
    nc._exp_engines = exp_engines

    ctr = [0]

    def nm(pfx):
        ctr[0] += 1
        return f"{pfx}_{ctr[0]}"

    # est-clock cost constants (ns, TimelineSim calibrated)
    DR512 = 109.0         # one DR matmul, 512 out cols
    DR128 = 29.0          # one DR matmul, 128 out cols
    BF512 = 216.0         # one bf16 matmul, 512 out cols
    EXP_ACT = 1040.0
    EXP_DVE = 1240.0
    CONV_DVE = 703.0      # [128,512] psum->sbuf convert on DVE
    CONV_ACT = 601.0
    ATTNV_NS = {"split8": 64 * 15.7, "plain8": 32 * 15.7, "bf16": 64 * 29.3}[attnv]
    TRANSP = 222.0
    NORM = 612.0          # recip + bcast mult on DVE

    with tile.TileContext(nc) as tc:
        with tc.tile_pool(name="persist", bufs=1) as pp, \
             tc.tile_pool(name="work", bufs=1) as wp, \
             tc.tile_pool(name="ps", bufs=1, space="PSUM") as ps:
            # ---- persistent SBUF ----
            x_hi = pp.tile([128, NKT, S], fp8)
            x_lo = pp.tile([128, NKT, S], fp8)
            x_md = pp.tile([128, NKT, S], fp8)
            eye = pp.tile([128, 128], bf16)
            bqs = pp.tile([128, NPAIR], f32)
            bks = pp.tile([128, NPAIR], f32)
            wo_sb = pp.tile([128, NPAIR, DIM], bf16)
            onorm = pp.tile([128, NPAIR, S], bf16)   # [(j,e), pair, s]

            est = {"pe": 0.0, "act": 0.0, "dve": 0.0}

            def flex(cost_act, cost_dve, fn_act, fn_dve):
                """Emit a convert/copy on the engine with the lower est clock."""
                if est["act"] + cost_act <= est["dve"] + cost_dve:
                    fn_act()
                    est["act"] += cost_act
                else:
                    fn_dve()
                    est["dve"] += cost_dve

            xh_src = xh_d.ap().rearrange("(kt p) s -> p kt s", p=128)
            xl_src = xl_d.ap().rearrange("(kt p) s -> p kt s", p=128)
            xm_src = xm_d.ap().rearrange("(kt p) s -> p kt s", p=128)

            wtiles = {}

            def w_dmas(p, only=None):
                for pfx in ("k", "q", "v"):
                    if only and pfx not in only:
                        continue
                    for hl in ("h", "l"):
                        w = wp.tile([128, NKT, 128], fp8, tag=f"w{pfx}{hl}",
                                    bufs=2, name=nm(f"w{pfx}{hl}{p}"))
                        nc.sync.dma_start(out=w, in_=w_d[pfx + hl].ap()[p])
                        wtiles[(pfx, hl, p)] = w

            qkts = {}

            def pair_tiles(p):
                qT = wp.tile([128, S], fp8, tag="qT", bufs=2, name=nm(f"qT{p}"))
                kT = wp.tile([128, S], fp8, tag="kT", bufs=2, name=nm(f"kT{p}"))
                qdr = wp.tile([64, 2, S], fp8, tag="qdr", bufs=2,
                              name=nm(f"qdr{p}"))
                kdr = wp.tile([64, 2, S], fp8, tag="kdr", bufs=2,
                              name=nm(f"kdr{p}"))
                if att8:
                    vA = wp.tile([128, NT, NHL, 2, VW], fp8, tag="vA", bufs=2,
                                 name=nm(f"vA{p}"))
                else:
                    vA = wp.tile([128, NT, 1, 2, 72], bf16, tag="vA", bufs=2,
                                 name=nm(f"vA{p}"))
                qkts[p] = (qT, kT, qdr, kdr, vA)

            def ones_atom(p):
                def go():
                    vA = qkts[p][4]
                    nc.gpsimd.memset(vA[:, :, 0, :, 64:65], 1.0)
                    if split:
                        nc.gpsimd.memset(vA[:, :, 1, :, 64:65], 0.0)
                    if debug_taps and p == 0:
                        for hl in range(NHL):
                            for tt in range(NT):
                                nc.sync.dma_start(
                                    out=dbg["vA1"].ap()[:, tt, hl, :],
                                    in_=vA[:, tt, hl, :, 64])
                return go

            def proj_mms(psum, pfx, p, cols, x_cols):
                """12 DR matmuls: 3 chains x 4 kt-pairs into psum[128, n]."""
                wh, wl = wtiles[(pfx, "h", p)], wtiles[(pfx, "l", p)]
                chains = [(wh, x_hi), (wh, x_lo), (wl, x_md)]
                n = len(chains)
                for r in range(NKP):
                    for ci, (w, xs) in enumerate(chains):
                        nc.tensor.matmul(
                            psum, w[:, 2 * r:2 * r + 2, :],
                            xs[:, 2 * r:2 * r + 2, x_cols],
                            start=(r == 0 and ci == 0),
                            stop=(r == NKP - 1 and ci == n - 1),
                            perf_mode=DRM)

            def qk_convert(pfx, p, sq, psum):
                """+bias, ->fp8 qT/kT chunk, then reshuffle DMA chunk."""
                dst, ddr, bias = ((qkts[p][0], qkts[p][2], bqs) if pfx == "q"
                                  else (qkts[p][1], qkts[p][3], bks))
                cols = slice(sq * 512, (sq + 1) * 512)

                def on_dve():
                    nc.vector.tensor_scalar_add(
                        out=dst[:, cols], in0=psum, scalar1=bias[:, p:p + 1])

                def on_act():
                    nc.scalar.activation(dst[:, cols], psum, AF.Identity,
                                         bias=bias[:, p:p + 1])
                flex(CONV_ACT, CONV_DVE, on_act, on_dve)
                if debug_taps and p == 0:
                    nc.sync.dma_start(out=dbg["qT" if pfx == "q" else "kT"]
                                      .ap()[:, cols], in_=dst[:, cols])
                for latom in range(2):
                    nc.sync.dma_start(out=ddr[:, latom, cols],
                                      in_=dst[latom::2, cols])
                if debug_taps and p == 0:
                    nc.sync.dma_start(out=dbg["qdr" if pfx == "q" else "kdr"]
                                      .ap()[:, :, cols], in_=ddr[:, :, cols])

            def qk_atoms(pfx, p, sq):
                """Two PE atoms (6 DR matmuls each) + convert + reshuffle."""
                cell = {}
                cols = slice(sq * 512, (sq + 1) * 512)

                def steps():
                    wh, wl = wtiles[(pfx, "h", p)], wtiles[(pfx, "l", p)]
                    return [(r, w, xs) for r in range(NKP)
                            for (w, xs) in ((wh, x_hi), (wh, x_lo),
                                            (wl, x_md))]

                def a1():
                    cell["ps"] = ps.tile([128, 512], f32, tag="qk", bufs=1,
                                         name=nm(f"ps{pfx}{p}{sq}"))
                    for mm, (r, w, xs) in enumerate(steps()[:6]):
                        nc.tensor.matmul(
                            cell["ps"], w[:, 2 * r:2 * r + 2, :],
                            xs[:, 2 * r:2 * r + 2, cols],
                            start=(mm == 0), stop=False, perf_mode=DRM)

                def a2():
                    for mm, (r, w, xs) in enumerate(steps()[6:]):
                        nc.tensor.matmul(
                            cell["ps"], w[:, 2 * r:2 * r + 2, :],
                            xs[:, 2 * r:2 * r + 2, cols],
                            start=False, stop=(mm == 5), perf_mode=DRM)
                    qk_convert(pfx, p, sq, cell["ps"])
                return [a1, a2]

            def v_convert(p, ttg, psum):
                """v psum [128,4tt,128=(j,e)] -> vA hi (and lo residual)."""
                vA = qkts[p][4]
                src = psum.rearrange("p (t j e) -> p t j e", t=4, j=2)
                tts = slice(ttg * 4, (ttg + 1) * 4)

                def hi_dve():
                    nc.vector.tensor_copy(vA[:, tts, 0, :, 0:64], src)

                def hi_act():
                    nc.scalar.activation(vA[:, tts, 0, :, 0:64], src, AF.Copy)
                flex(CONV_ACT, CONV_DVE, hi_act, hi_dve)
                if split:
                    nc.vector.tensor_tensor(
                        out=vA[:, tts, 1, :, 0:64], in0=src,
                        in1=vA[:, tts, 0, :, 0:64],
                        op=mybir.AluOpType.subtract)
                    est["dve"] += CONV_DVE
                if debug_taps and p == 0:
                    for hl in range(NHL):
                        for jj in range(2):
                            nc.sync.dma_start(
                                out=dbg["vA"].ap()[:, tts, hl, jj, 0:64],
                                in_=vA[:, tts, hl, jj, 0:64])

            def v_atoms(p, ttg):
                """Two PE atoms (2 t-tiles of 12 DR MMs each) + convert."""
                cell = {}

                def half(h):
                    def go():
                        if h == 0:
                            cell["ps"] = ps.tile([128, 512], f32, tag="vps",
                                                 bufs=1, name=nm(f"psv{p}{ttg}"))
                        v_ps4 = cell["ps"].rearrange("p (t m) -> p t m", t=4)
                        wh, wl = wtiles[("v", "h", p)], wtiles[("v", "l", p)]
                        for t4 in (0, 1) if h == 0 else (2, 3):
                            tt = ttg * 4 + t4
                            cols = slice(tt * 128, (tt + 1) * 128)
                            mm = 0
                            for r in range(NKP):
                                for w, xs in ((wh, x_hi), (wh, x_lo),
                                              (wl, x_md)):
                                    nc.tensor.matmul(
                                        v_ps4[:, t4, :],
                                        xs[:, 2 * r:2 * r + 2, cols],
                                        w[:, 2 * r:2 * r + 2, :],
                                        start=(mm == 0), stop=(mm == 11),
                                        perf_mode=DRM)
                                    mm += 1
                        if h == 1:
                            v_convert(p, ttg, cell["ps"])
                    return go
                return [half(0), half(1)]

            def qkv_atoms(p):
                """(resource_key, cost_ns, closure) atoms for pair p."""
                pair_tiles(p)
                out = [((p, "v"), 100, ones_atom(p))]
                for ttg in range(4):
                    out += [((p, "v"), 700, a) for a in v_atoms(p, ttg)]
                for sq in range(NSQ):
                    out += [((p, "k"), 660, a) for a in qk_atoms("k", p, sq)]
                for sq in range(NSQ):
                    out += [((p, f"q{sq}"), 660, a)
                            for a in qk_atoms("q", p, sq)]
                return out

            GREEDY_M = greedy_m
            exp_hist = [0.0, 0.0]   # completion time per sc-tile rotation

            hot = deque()
            cold = deque()

            def pop_cold_one():
                key, cost, fn = cold.popleft()
                fn()
                est["pe"] += cost

            def force_res(*keys):
                while any(k for (k, c, f) in cold if k in keys):
                    pop_cold_one()

            def greedy_cold():
                while cold and (est["pe"] + cold[0][1] + 2 * DR512
                                <= max(est["act"], est["dve"]) + GREEDY_M):
                    pop_cold_one()

            def oproj_st(st):
                out = []
                for nh in range(2):
                    def go(st=st, nh=nh):
                        tag = "qk" if nh == 0 else "vps"
                        y_ps = ps.tile([128, 512], f32, tag=tag, bufs=1,
                                       name=nm(f"psy{st}{nh}"))
                        for p in range(NPAIR):
                            nc.tensor.matmul(
                                y_ps,
                                onorm[:, p, st * 128:(st + 1) * 128],
                                wo_sb[:, p, nh * 512:(nh + 1) * 512],
                                start=(p == 0), stop=(p == NPAIR - 1))
                        y_sb = wp.tile([128, 512], f32, tag="ysb",
                                       bufs=4, name=nm(f"ysb{st}{nh}"))

                        def y_dve():
                            nc.vector.tensor_copy(y_sb, y_ps)

                        def y_act():
                            nc.scalar.activation(y_sb, y_ps, AF.Copy)
                        flex(CONV_ACT, CONV_DVE, y_act, y_dve)
                        nc.sync.dma_start(
                            out=y_d.ap()[st * 128:(st + 1) * 128,
                                         nh * 512:(nh + 1) * 512],
                            in_=y_sb)
                    out.append(("oproj", 880, go))
                return out

            def exp_group(att_t, sc, g):
                """exp of one [128,2,512] scores psum -> att tile; pick engine."""
                dst = att_t[:, 2 * g:2 * g + 2, :]
                use_act = (est["act"] + EXP_ACT
                           <= est["dve"] + EXP_DVE + exp_margin)
                exp_engines.append(1 if use_act else 0)
                if use_act:
                    nc.scalar.activation(dst, sc, AF.Exp, scale=SCALE)
                    est["act"] = max(est["act"], est["pe"] + 150) + EXP_ACT
                    done = est["act"]
                else:
                    if att8:
                        nc.vector.tensor_scalar(
                            out=dst.bitcast(u8), in0=sc, scalar1=A8,
                            scalar2=B8, op0=mybir.AluOpType.mult,
                            op1=mybir.AluOpType.add)
                    else:
                        nc.vector.tensor_scalar(
                            out=dst.bitcast(u16), in0=sc, scalar1=A16,
                            scalar2=B16, op0=mybir.AluOpType.mult,
                            op1=mybir.AluOpType.add)
                    est["dve"] = max(est["dve"], est["pe"] + 150) + EXP_DVE
                    done = est["dve"]
                exp_hist.append(done)

            def attn_back(p, j, sq, att_t, last_slot):
                """attn@v + normalize + transpose (+ onorm copy & oproj)."""
                def go():
                    vA = qkts[p][4]
                    o_ps = ps.tile([128, 4, 65], f32, tag="ops", bufs=1,
                                   name=nm(f"o{p}{j}{sq}"))
                    for sti in range(4):
                        scols = slice(sti * 128, (sti + 1) * 128)
                        for gg in range(NT // 2):
                            tts = slice(2 * gg, 2 * gg + 2)
                            for hl in range(NHL):
                                if att8:
                                    nc.tensor.matmul(
                                        o_ps[:, sti, :],
                                        att_t[:, tts, scols],
                                        vA[:, tts, hl, j, 0:65],
                                        start=(gg == 0 and hl == 0),
                                        stop=(gg == NT // 2 - 1
                                              and hl == NHL - 1),
                                        perf_mode=DRM)
                                else:
                                    for ti in range(2):
                                        tt = 2 * gg + ti
                                        nc.tensor.matmul(
                                            o_ps[:, sti, :],
                                            att_t[:, tt, scols],
                                            vA[:, tt, 0, j, 0:65],
                                            start=(gg == 0 and ti == 0),
                                            stop=(gg == NT // 2 - 1
                                                  and ti == 1))
                    rinv = wp.tile([128, 4], f32, tag="rinv", bufs=2,
                                   name=nm(f"ri{p}{j}{sq}"))
                    nc.vector.reciprocal(rinv, o_ps[:, :, 64])
                    o_n = wp.tile([128, 4, 64], bf16, tag="on", bufs=2,
                                  name=nm(f"on{p}{j}{sq}"))
                    nc.vector.tensor_tensor(
                        out=o_n, in0=o_ps[:, :, 0:64],
                        in1=rinv[:, :, None].broadcast_to([128, 4, 64]),
                        op=mybir.AluOpType.mult)
                    est["dve"] += NORM
                    pt = pt_for[(p, sq)]
                    for sti in range(4):
                        nc.tensor.matmul(
                            pt[64 * j:64 * (j + 1), sti, :],
                            o_n[:, sti, :], eye, is_transpose=True)
                    if j == 1:
                        def cp_dve():
                            nc.vector.tensor_copy(
                                onorm[:, p, sq * 512:(sq + 1) * 512],
                                pt.rearrange("p st s -> p (st s)"))

                        def cp_act():
                            nc.scalar.activation(
                                onorm[:, p, sq * 512:(sq + 1) * 512],
                                pt.rearrange("p st s -> p (st s)"), AF.Copy)
                        flex(CONV_ACT, 437.0, cp_act, cp_dve)
                        if last_slot:
                            for sti in range(4):
                                cold.extend(oproj_st(sq * 4 + sti))
                return go

            def startup_pair0():
                """kt-pair-outer k-proj + q(sq0) emitted per arriving x chunk;
                v and q sq1-3 to cold."""
                pair_tiles(0)
                kps = [ps.tile([128, 2, 512], f32, tag="sc", bufs=2,
                               name=nm(f"kp0{i}")) for i in range(2)]
                q_ps = ps.tile([128, 512], f32, tag="qk", bufs=1,
                               name=nm("psq00"))
                wkh, wkl = wtiles[("k", "h", 0)], wtiles[("k", "l", 0)]
                wqh, wql = wtiles[("q", "h", 0)], wtiles[("q", "l", 0)]
                for r in range(NKP + 1):
                    if r < NKP:
                        for ci, (w, xs) in enumerate(
                                ((wkh, x_hi), (wkh, x_lo), (wkl, x_md))):
                            st_ = r == 0 and ci == 0
                            sp_ = r == NKP - 1 and ci == 2
                            for sqh in range(NSQ):
                                nc.tensor.matmul(
                                    kps[sqh // 2][:, sqh % 2,
                                                  :], w[:, 2 * r:2 * r + 2, :],
                                    xs[:, 2 * r:2 * r + 2,
                                       sqh * 512:(sqh + 1) * 512],
                                    start=st_, stop=sp_, perf_mode=DRM)
                    if r >= 1:
                        rq = r - 1
                        for ci, (w, xs) in enumerate(
                                ((wqh, x_hi), (wqh, x_lo), (wql, x_md))):
                            nc.tensor.matmul(
                                q_ps, w[:, 2 * rq:2 * rq + 2, :],
                                xs[:, 2 * rq:2 * rq + 2, 0:512],
                                start=(rq == 0 and ci == 0),
                                stop=(rq == NKP - 1 and ci == 2),
                                perf_mode=DRM)

                # converts + reshuffles: k chunks then q chunk 0 (DVE to keep
                # ACT's pipeline clean at startup is not needed; flex is fine)
                for sqh in range(NSQ):
                    qk_convert("k", 0, sqh, kps[sqh // 2][:, sqh % 2, :])
                qk_convert("q", 0, 0, q_ps)
                for ttg in range(4):
                    cold.extend(((0, "v"), 700, a) for a in v_atoms(0, ttg))
                cold.appendleft(((0, "v"), 100, ones_atom(0)))
                for sq in range(1, NSQ):
                    cold.extend(((0, f"q{sq}"), 660, a)
                                for a in qk_atoms("q", 0, sq))
                est["pe"] = 18000.0
                est["act"] = 18000.0
                est["dve"] = 18000.0

            pt_for = {}

            w_dmas(0, only=("k",))
            nc.sync.dma_start(out=bqs, in_=bq_d.ap())
            nc.sync.dma_start(out=bks, in_=bk_d.ap())
            nc.sync.dma_start(out=eye, in_=eye_d.ap())
            for kt in range(NKT):
                nc.sync.dma_start(out=x_hi[:, kt, :], in_=xh_src[:, kt, :])
                nc.sync.dma_start(out=x_lo[:, kt, :], in_=xl_src[:, kt, :])
                nc.sync.dma_start(out=x_md[:, kt, :], in_=xm_src[:, kt, :])
                if kt == 1:
                    w_dmas(0, only=("q", "v"))
            nc.sync.dma_start(out=wo_sb, in_=wo_d.ap())
            startup_pair0()
            for slot in range(1, NPAIR + 1):
                if slot < NPAIR:
                    w_dmas(slot)
                    cold.extend(qkv_atoms(slot))
                p = slot - 1
                last_slot = slot == NPAIR
                for sq in range(NSQ):
                    pt_for[(p, sq)] = ps.tile(
                        [128, 4, 128], bf16, tag="pt", bufs=1,
                        name=nm(f"pt{p}{sq}"))
                    for j in range(2):
                        force_res((p, "k"), (p, f"q{sq}"))
                        qT, kT, qdr, kdr, vA = qkts[p]
                        att_t = wp.tile([128, NT, 512], att_dt, tag="att",
                                        bufs=3, name=nm(f"att{p}{j}{sq}"))
                        for g in range(NT // 2):
                            sc = ps.tile([128, 2, 512], f32, tag="sc",
                                         bufs=2, name=nm(f"sc{p}{j}{sq}{g}"))
                            dep = exp_hist[-2]
                            for i in range(2):
                                tt = 2 * g + i
                                nc.tensor.matmul(
                                    sc[:, i, :],
                                    kdr[32 * j:32 * j + 32, :,
                                        tt * 128:(tt + 1) * 128],
                                    qdr[32 * j:32 * j + 32, :,
                                        sq * 512:(sq + 1) * 512],
                                    start=True, stop=True, perf_mode=DRM)
                            est["pe"] = max(est["pe"] + 2 * DR512,
                                            dep + 150 + 2 * DR512)
                            exp_group(att_t, sc, g)
                            if g == 3 and hot:
                                hp, hfn = hot.popleft()
                                force_res((hp, "v"))
                                hfn()
                                if len(exp_hist) >= 5:
                                    est["pe"] = max(est["pe"],
                                                    exp_hist[-5] + 150)
                                est["pe"] += ATTNV_NS + TRANSP
                            else:
                                greedy_cold()
                        hot.append((p, attn_back(p, j, sq, att_t, last_slot)))
                        if debug_taps and p == 0 and sq == 0:
                            nc.sync.dma_start(out=dbg["att1" if j else "att0"]
                                              .ap(), in_=att_t)
            while hot:
                hp, hfn = hot.popleft()
                force_res((hp, "v"))
                hfn()
            while cold:
                pop_cold_one()
            if debug_taps:
                nc.sync.dma_start(out=dbg["on"].ap(), in_=onorm)
    nc.compile()
    return nc


def _get_nc():
    if "nc" not in _CACHE:
        _CACHE["nc"] = _build()
    return _CACHE["nc"]


def _f8(a):
    import ml_dtypes
    return np.ascontiguousarray(a).astype(ml_dtypes.float8_e4m3)


def _bf16(a):
    import ml_dtypes
    return np.ascontiguousarray(a).astype(ml_dtypes.bfloat16)


def make_in_maps(x, Wq, Wk, Wv, bq, bk, bv, Wo, bo):
    eye = np.eye(128, dtype=np.float32)
    in_maps = []
    for c in range(NCORES):
        b, g = c // 2, c % 2
        hs = slice(g * HPC, (g + 1) * HPC)
        # weights pair-packed + sbuf-contiguous: [pair, 128pp, NKT, 128=(j,e)]
        # lo residual stored x32 (raw residuals underflow fp8 subnormals);
        # paired with xm = fp8(x/32) so the scales cancel in chain 3.
        def wprep(W):
            w3 = W[hs].reshape(NPAIR, 2, DIM, DH).transpose(0, 2, 1, 3) \
                .reshape(NPAIR, NKT, 128, 128).transpose(0, 2, 1, 3)
            w3 = np.ascontiguousarray(w3)
            whi = _f8(w3)
            wlo = _f8(32.0 * (w3 - whi.astype(np.float32)))
            return whi, wlo
        wqh, wql = wprep(Wq)
        wkh, wkl = wprep(Wk)
        wvh, wvl = wprep(Wv)
        xT = np.ascontiguousarray(x[b].T)
        xh = _f8(xT)
        xl = _f8(xT - xh.astype(np.float32))
        xm = _f8(xT / 32.0)
        # wo: [128=(j,e), pair, DIM]
        wo3 = Wo[g * 512:(g + 1) * 512, :].reshape(NPAIR, 128, DIM) \
            .transpose(1, 0, 2)
        in_maps.append({
            "xh": xh, "xl": xl, "xm": xm,
            "wqh": wqh, "wql": wql,
            "wkh": wkh, "wkl": wkl,
            "wvh": wvh, "wvl": wvl,
            "bq": np.ascontiguousarray(bq[hs].reshape(NPAIR, 128).T),
            "bk": np.ascontiguousarray(bk[hs].reshape(NPAIR, 128).T),
            "wo": _bf16(wo3),
            "eye": _bf16(eye),
        })
    return in_maps


def combine(results, bv, Wo, bo):
    const = bv.reshape(DIM) @ Wo + bo          # [DIM]
    y = np.empty((B, S, DIM), dtype=np.float32)
    for b in range(B):
        y[b] = (results[2 * b]["y"].astype(np.float32)
                + results[2 * b + 1]["y"].astype(np.float32) + const)
    return y


def kernel(x, Wq, Wk, Wv, bq, bk, bv, Wo, bo):
    import time
    from concourse.bass_utils import run_bass_kernel_spmd
    x, Wq, Wk, Wv, bq, bk, bv, Wo, bo = [
        np.asarray(a, dtype=np.float32)
        for a in (x, Wq, Wk, Wv, bq, bk, bv, Wo, bo)]
    nc = _get_nc()
    in_maps = make_in_maps(x, Wq, Wk, Wv, bq, bk, bv, Wo, bo)
    last = None
    for attempt in range(3):
        try:
            res = run_bass_kernel_spmd(nc, in_maps,
                                       core_ids=list(range(NCORES)))
            return combine(res.results, bv, Wo, bo)
        except Exception as e:  # transient NRT_EXEC_UNIT_UNRECOVERABLE wedges
            last = e
            time.sleep(75)
    raise last
